# revision 3
# baseline (speedup 1.0000x reference)
import numpy as np

# Sliding-window min: out[t] = min(padded[t .. t+255]), padded = signal ++ 256*[signal[-1]]
# T = 1e6 elements sharded over 8 NeuronCores, 131072 outputs per core laid out as
# [128 partitions, 1024 cols]; each partition row is a contiguous 1280-element chunk
# (1024 outputs + 256 halo). Van Herk / Gil-Werman: per-256-block prefix min P and
# suffix min S (tensor_tensor_scan with masked reset), then out[f] = min(S[f], P[f+255]).

T = 1_000_000
W = 256
NCORES = 8
ROWS = 128
F = 1024
RW = F + W          # 1280
C = ROWS * F        # 131072 outputs per core
NEG = -3.0e38


def _build_bass():
    import concourse.bass as bass
    from concourse import mybir

    nc = bass.Bass()
    f32 = mybir.dt.float32
    x_ext = nc.declare_dram_parameter("x", [ROWS, RW], f32, isOutput=False)
    out_ext = nc.declare_dram_parameter("out", [ROWS, F], f32, isOutput=True)

    x = nc.alloc_sbuf_tensor("x_sb", [ROWS, RW], f32)
    mp = nc.alloc_sbuf_tensor("mp_sb", [ROWS, RW], f32)
    ms = nc.alloc_sbuf_tensor("ms_sb", [ROWS, RW], f32)
    P = nc.alloc_sbuf_tensor("p_sb", [ROWS, RW], f32)
    S = nc.alloc_sbuf_tensor("s_sb", [ROWS, RW], f32)
    o = nc.alloc_sbuf_tensor("o_sb", [ROWS, F], f32)

    dsem = nc.alloc_semaphore("dsem")
    gsem = nc.alloc_semaphore("gsem")
    asem = nc.alloc_semaphore("asem")
    vsem = nc.alloc_semaphore("vsem")

    mn = mybir.AluOpType.min
    mx = mybir.AluOpType.max

    with nc.Block() as block:

        @block.sync
        def _(sync):
            sync.dma_start(out=x[:], in_=x_ext[:]).then_inc(dsem, 16)
            sync.wait_ge(vsem, 1)
            sync.dma_start(out=out_ext[:], in_=o[:]).then_inc(dsem, 16)
            sync.wait_ge(dsem, 32)

        @block.gpsimd
        def _(g):
            g.memset(mp[:], NEG).then_inc(gsem, 1)
            g.memset(ms[:], NEG).then_inc(gsem, 1)

        @block.scalar
        def _(act):
            act.wait_ge(dsem, 16)
            act.wait_ge(gsem, 1)
            # mask = x at block-reset columns, -BIG elsewhere
            act.copy(mp[:, 0:RW:W], x[:, 0:RW:W]).then_inc(asem, 1)
            act.wait_ge(gsem, 2)
            act.copy(ms[:, W - 1:RW:W], x[:, W - 1:RW:W]).then_inc(asem, 1)

        @block.vector
        def _(v):
            v.wait_ge(dsem, 16)
            v.wait_ge(asem, 1)
            # state = max(min(x, state), mask): running min, reset to x where mask==x
            v.tensor_tensor_scan(P[:], x[:], mp[:], 0.0, mn, mx)
            v.wait_ge(asem, 2)
            v.tensor_tensor_scan(
                S[:, ::-1], x[:, ::-1], ms[:, ::-1], 0.0, mn, mx
            )
            v.drain()
            v.tensor_tensor(
                o[:], S[:, 0:F], P[:, W - 1:W - 1 + F], mn
            ).then_inc(vsem, 1)

    return nc


def _shard_inputs(signal: np.ndarray):
    sig = np.ascontiguousarray(signal, dtype=np.float32)
    pad_val = sig[-1]
    need = (NCORES - 1) * C + (ROWS - 1) * F + RW
    padded = np.empty(need, dtype=np.float32)
    padded[:T] = sig
    padded[T:] = pad_val
    in_maps = []
    for i in range(NCORES):
        v = np.lib.stride_tricks.as_strided(
            padded[i * C:], shape=(ROWS, RW), strides=(4 * F, 4)
        )
        in_maps.append({"x": np.ascontiguousarray(v)})
    return in_maps


def kernel(signal: np.ndarray) -> np.ndarray:
    from concourse.bass_utils import run_bass_kernel_spmd

    nc = _build_bass()
    in_maps = _shard_inputs(signal)
    res = run_bass_kernel_spmd(nc, in_maps, core_ids=list(range(NCORES)))
    outs = [r["out"].reshape(-1) for r in res.results]
    return np.concatenate(outs)[:T].astype(np.float32)


# revision 8
# speedup vs baseline: 1.3470x; 1.3470x over previous
import numpy as np

# Sliding-window min: out[t] = min(padded[t .. t+255]), padded = signal ++ 256*[signal[-1]]
# T = 1e6 elements sharded over 8 NeuronCores, 131072 outputs per core laid out as
# [128 partitions, 1024 cols]; each partition row is a contiguous 1280-element chunk
# (1024 outputs + 256 halo). Van Herk / Gil-Werman with 256-blocks per row:
#   P[f] = prefix min within f's block, S[f] = suffix min within f's block
#   out[f] = min(S[f], P[f+255]); out[0] = S[0], so the P scan skips block 0.
# P/S via tensor_tensor_scan (op0=min, op1=max with a reset mask: mask==x at block
# starts resp. ends, -BIG elsewhere). Two column chunks pipeline DVE scans with
# GPSIMD combines; DMAs are split across both HWDGE rings (sync + scalar) by rows.

T = 1_000_000
W = 256
NCORES = 8
ROWS = 128
F = 1024
RW = F + W          # 1280
C = ROWS * F        # 131072 outputs per core
NEG = -3.0e38


def _build_bass():
    import concourse.bass as bass
    from concourse import mybir

    nc = bass.Bass()
    f32 = mybir.dt.float32
    x_ext = nc.declare_dram_parameter("x", [ROWS, RW], f32, isOutput=False)
    out_ext = nc.declare_dram_parameter("out", [ROWS, F], f32, isOutput=True)

    x = nc.alloc_sbuf_tensor("x_sb", [ROWS, RW], f32)
    mp = nc.alloc_sbuf_tensor("mp_sb", [ROWS, RW], f32)
    ms = nc.alloc_sbuf_tensor("ms_sb", [ROWS, RW], f32)
    P = nc.alloc_sbuf_tensor("p_sb", [ROWS, RW], f32)
    S = nc.alloc_sbuf_tensor("s_sb", [ROWS, RW], f32)
    o = nc.alloc_sbuf_tensor("o_sb", [ROWS, F], f32)

    dsemA = nc.alloc_semaphore("dsemA")  # input DMAs chunk A (2 x 16)
    dsemB = nc.alloc_semaphore("dsemB")  # input DMAs chunk B (2 x 16)
    gsem = nc.alloc_semaphore("gsem")    # gpsimd memsets + mask copies
    vsem = nc.alloc_semaphore("vsem")    # DVE scan completions
    csem = nc.alloc_semaphore("csem")    # DVE combine completions
    zsem = nc.alloc_semaphore("zsem")    # gpsimd out[0] copy
    osem = nc.alloc_semaphore("osem")    # output DMAs

    mn = mybir.AluOpType.min
    mx = mybir.AluOpType.max

    HR = ROWS // 2  # row split for dual-ring DMA
    CA = 768        # input column chunk A = [0, 768), B = [768, 1280)

    with nc.Block() as block:

        @block.sync
        def _(sync):
            sync.dma_start(out=x[0:HR, 0:CA], in_=x_ext[0:HR, 0:CA]).then_inc(dsemA, 16)
            sync.dma_start(out=x[0:HR, CA:RW], in_=x_ext[0:HR, CA:RW]).then_inc(dsemB, 16)
            sync.wait_ge(zsem, 1)
            sync.wait_ge(csem, 1)
            sync.dma_start(out=out_ext[0:HR, 0:512], in_=o[0:HR, 0:512]).then_inc(osem, 16)
            sync.wait_ge(csem, 2)
            sync.dma_start(out=out_ext[0:HR, 512:F], in_=o[0:HR, 512:F]).then_inc(osem, 16)
            sync.wait_ge(osem, 64)

        @block.scalar
        def _(act):
            act.dma_start(out=x[HR:ROWS, 0:CA], in_=x_ext[HR:ROWS, 0:CA]).then_inc(dsemA, 16)
            act.dma_start(out=x[HR:ROWS, CA:RW], in_=x_ext[HR:ROWS, CA:RW]).then_inc(dsemB, 16)
            act.wait_ge(zsem, 1)
            act.wait_ge(csem, 1)
            act.dma_start(out=out_ext[HR:ROWS, 0:512], in_=o[HR:ROWS, 0:512]).then_inc(osem, 16)
            act.wait_ge(csem, 2)
            act.dma_start(out=out_ext[HR:ROWS, 512:F], in_=o[HR:ROWS, 512:F]).then_inc(osem, 16)

        @block.gpsimd
        def _(g):
            # masks: mp over blocks 1-4 (cols 256:1280), ms over blocks 0-3 (cols 0:1024)
            g.memset(mp[:, W:RW], NEG).then_inc(gsem, 1)
            g.memset(ms[:, 0:F], NEG).then_inc(gsem, 1)
            g.drain()
            g.wait_ge(dsemA, 32)
            g.tensor_copy(mp[:, W:CA:W], x[:, W:CA:W]).then_inc(gsem, 1)          # 256,512
            g.tensor_copy(ms[:, W - 1:CA:W], x[:, W - 1:CA:W]).then_inc(gsem, 1)  # 255,511,767
            g.wait_ge(dsemB, 32)
            g.tensor_copy(mp[:, CA:RW:W], x[:, CA:RW:W]).then_inc(gsem, 1)        # 768,1024
            g.tensor_copy(ms[:, F - 1:F:W], x[:, F - 1:F:W]).then_inc(gsem, 1)    # 1023
            # out[0] = S[0] (full block-0 min; P has no block 0)
            g.wait_ge(vsem, 2)  # P1 + S1
            g.tensor_copy(o[:, 0:1], S[:, 0:1]).then_inc(zsem, 1)

        @block.vector
        def _(v):
            v.wait_ge(gsem, 3)  # memsets + chunk A mp copy
            # P1: blocks 1-2 (cols 256:768); resets at 256, 512
            v.tensor_tensor_scan(
                P[:, W:CA], x[:, W:CA], mp[:, W:CA], 0.0, mn, mx
            ).then_inc(vsem, 1)
            v.wait_ge(gsem, 4)  # chunk A ms copy
            # S1: blocks 0-1 reversed (cols 511..0); resets at 511, 255
            v.tensor_tensor_scan(
                S[:, 511::-1], x[:, 511::-1], ms[:, 511::-1], 0.0, mn, mx
            ).then_inc(vsem, 1)
            # C1: out[1:512) = min(S[1:512), P[256:767))
            v.drain()
            v.tensor_tensor(
                o[:, 1:512], S[:, 1:512], P[:, W:W + 511], mn
            ).then_inc(csem, 1)
            v.wait_ge(gsem, 6)  # chunk B mask copies
            # P2: blocks 3-4 (cols 768:1280); resets at 768, 1024
            v.tensor_tensor_scan(
                P[:, CA:RW], x[:, CA:RW], mp[:, CA:RW], 0.0, mn, mx
            ).then_inc(vsem, 1)
            # S2: blocks 2-3 reversed (cols 1023..512); resets at 1023, 767
            v.tensor_tensor_scan(
                S[:, F - 1:511:-1], x[:, F - 1:511:-1], ms[:, F - 1:511:-1],
                0.0, mn, mx,
            ).then_inc(vsem, 1)
            # C2: out[512:1024) = min(S[512:1024), P[767:1279))
            v.drain()
            v.tensor_tensor(
                o[:, 512:F], S[:, 512:F], P[:, 512 + W - 1:F - 1 + W], mn
            ).then_inc(csem, 1)

    return nc


def _shard_inputs(signal: np.ndarray):
    sig = np.ascontiguousarray(signal, dtype=np.float32)
    pad_val = sig[-1]
    need = (NCORES - 1) * C + (ROWS - 1) * F + RW
    padded = np.empty(need, dtype=np.float32)
    padded[:T] = sig
    padded[T:] = pad_val
    in_maps = []
    for i in range(NCORES):
        v = np.lib.stride_tricks.as_strided(
            padded[i * C:], shape=(ROWS, RW), strides=(4 * F, 4)
        )
        in_maps.append({"x": np.ascontiguousarray(v)})
    return in_maps


def kernel(signal: np.ndarray) -> np.ndarray:
    from concourse.bass_utils import run_bass_kernel_spmd

    nc = _build_bass()
    in_maps = _shard_inputs(signal)
    res = run_bass_kernel_spmd(nc, in_maps, core_ids=list(range(NCORES)))
    outs = [r["out"].reshape(-1) for r in res.results]
    return np.concatenate(outs)[:T].astype(np.float32)
